# revision 7
# baseline (speedup 1.0000x reference)
"""
Trainium2 Bass kernel for DynamicGraphAttention
(softmax(Hn Wq^T (Hn Wk^T)^T / sqrt(D) + eta*logit(clip(A)) masked)).

Shapes (hardcoded):
  Hn     [16, 2048, 256] f32
  A_stat [2048, 2048]    f32
  M_mask [2048, 2048]    int32
  Wq, Wk [256, 256]      f32
  out    [16, 2048, 2048] f32

Sharding across 8 NeuronCores: 4 batch-groups x 2 seq(query)-groups
(minimizes per-core input DMA: 4MB HnT + 4MB E-table). Core c handles
batches of group bg = c // 2 and query rows [qg*1024:(qg+1)*1024]
(qg = c % 2). Key columns are ROTATED per core so the core's own query
block is always columns [0:1024] of its HnT copy -- that keeps the
program SPMD-uniform; the host un-rotates output columns on assembly.

Math: S = Hn G Hn^T with G = Wq^T Wk / sqrt(D) (host-computed, fp16).
p = exp(S) * E / rowsum(exp(S) * E), where E = M * A/(1-A) (clipped,
host-computed fp16) -- exp(s+logit(a)) == exp(s)*a/(1-a) exactly.

Device pipeline per (batch b, 128-row q-tile):
  VT[b] = G^T HnT[b,:, :1024]  (per batch, PE, fp16 via DVE copy)
  S     = VT^T HnT             PSUM f32, 8 matmuls    (PE)
  t     = exp(S)               fp16                   (ACT)
  u,rs  = t*E, rowsum          fp16 + f32 accum       (DVE ttr)
  out   = u * (1/rs)           bf16 -> DMA            (DVE)

Output is shipped bf16 (rel err ~0.4% << 2e-2 gate) halving out DMA.
Per-core DMA: in 4+4+0.13MB, out 16.8MB  (baseline moved ~50MB).
"""

import math

import numpy as np

import concourse.bass as bass
import concourse.bacc as bacc
import concourse.tile as tile
from concourse import mybir
from concourse import bass_utils

F32 = mybir.dt.float32
BF16 = mybir.dt.bfloat16
FP16 = mybir.dt.float16

B_FULL = 16
N = 2048
D = 256
NBG = 4   # batch groups
NQG = 2   # seq (query-row) groups
NB = B_FULL // NBG        # batches per core = 4
NQ = N // NQG             # query rows per core = 1024
NQT = NQ // 128           # q tiles per core-batch = 8
EPS = 1e-3
SCALE = 1.0 / math.sqrt(float(D))  # 1/16

_CACHE = {}


def _build():
    nc = bacc.Bacc("TRN2", debug=False, enable_asserts=False)

    hnt_d = nc.dram_tensor("hnt", [NB, D, N], FP16, kind="ExternalInput").ap()
    g_d = nc.dram_tensor("g", [D, D], FP16, kind="ExternalInput").ap()
    e_d = nc.dram_tensor("e", [NQ, N], FP16, kind="ExternalInput").ap()
    o_d = nc.dram_tensor("o", [NB, NQ, N], BF16, kind="ExternalOutput").ap()

    with tile.TileContext(nc) as tc:
        with (
            tc.tile_pool(name="consts", bufs=1) as consts,
            tc.tile_pool(name="vtp", bufs=4) as vtp,
            tc.tile_pool(name="tp", bufs=3) as tp,
            tc.tile_pool(name="up", bufs=3) as up,
            tc.tile_pool(name="op", bufs=4) as op,
            tc.tile_pool(name="rsp", bufs=8) as rsp,
            tc.tile_pool(name="ps_s", bufs=2, space="PSUM") as ps_s,
        ):
            # ---- constants / persistent inputs ----
            g_sb = consts.tile([128, 2, D], FP16, tag="g")
            nc.sync.dma_start(out=g_sb, in_=g_d.rearrange("(i p) e -> p i e", p=128))

            hnt = []
            for b in range(NB):
                h_b = consts.tile([128, 2, N], FP16, tag=f"hnt{b}", name=f"hnt{b}")
                nc.sync.dma_start(
                    out=h_b, in_=hnt_d[b].rearrange("(i p) n -> p i n", p=128)
                )
                hnt.append(h_b)

            etab = []
            for t in range(NQT):
                e_t = consts.tile([128, N], FP16, tag=f"et{t}", name=f"et{t}")
                nc.gpsimd.dma_start(out=e_t, in_=e_d[t * 128:(t + 1) * 128, :])
                etab.append(e_t)

            # ---- per-batch: VT = G^T HnT[:, :NQ]  ([128, 2*NQ] fp16,
            #      cols j*NQ+q hold (G^T HnT)[j*128+e, q]) ----
            def emit_vt(b):
                vp = ps_s.tile([128, N], F32, tag="s", name=f"vp{b}")
                for j in range(2):
                    for c in range(NQ // 512):
                        sl = slice(j * NQ + c * 512, j * NQ + c * 512 + 512)
                        qsl = slice(c * 512, (c + 1) * 512)
                        for i in range(2):
                            nc.tensor.matmul(
                                vp[:, sl],
                                lhsT=g_sb[:, i, j * 128:(j + 1) * 128],
                                rhs=hnt[b][:, i, qsl],
                                start=(i == 0),
                                stop=(i == 1),
                            )
                vt_b = vtp.tile([128, N], FP16, tag="vt", name=f"vt{b}")
                # psum->sbuf copy on ACT (Copy+Exp share one table set;
                # keeps DVE free for the softmax epilogue)
                nc.scalar.copy(out=vt_b, in_=vp)
                return vt_b

            def emit_qtile(b, qt, vt_b, out_queue):
                qsl = slice(qt * 128, (qt + 1) * 128)
                s_ps = ps_s.tile([128, N], F32, tag="s", name=f"s{b}{qt}")
                for c in range(4):
                    csl = slice(c * 512, (c + 1) * 512)
                    for j in range(2):
                        nc.tensor.matmul(
                            s_ps[:, csl],
                            lhsT=vt_b[:, j * NQ + qt * 128:j * NQ + qt * 128 + 128],
                            rhs=hnt[b][:, j, csl],
                            start=(j == 0),
                            stop=(j == 1),
                        )
                t_sb = tp.tile([128, N], FP16, tag="t", name=f"t{b}{qt}")
                nc.scalar.activation(
                    out=t_sb, in_=s_ps,
                    func=mybir.ActivationFunctionType.Exp,
                )
                u_sb = up.tile([128, N], FP16, tag="u", name=f"u{b}{qt}")
                rs = rsp.tile([128, 1], F32, tag="rs", name=f"rs{b}{qt}")
                # u = (t * 1.0) * E with fused row-sum accumulator.
                # (InstTensorTensorReduce crashes TRN2 at runtime; this
                # InstTensorScalarPtr form is HW-verified.)
                nc.vector.scalar_tensor_tensor(
                    out=u_sb, in0=t_sb, scalar=1.0, in1=etab[qt],
                    op0=mybir.AluOpType.mult, op1=mybir.AluOpType.mult,
                    accum_out=rs,
                )
                rinv = rsp.tile([128, 1], F32, tag="rinv", name=f"ri{b}{qt}")
                nc.vector.reciprocal(out=rinv, in_=rs)
                o_sb = op.tile([128, N], BF16, tag="o", name=f"o{b}{qt}")
                nc.vector.tensor_scalar(
                    out=o_sb, in0=u_sb, scalar1=rinv, scalar2=None,
                    op0=mybir.AluOpType.mult,
                )
                out_queue.dma_start(out=o_d[b, qsl, :], in_=o_sb)

            # all VT up front: overlaps the input-DMA ramp, then the
            # S-tile stream owns both PSUM buffers uninterrupted
            vts = [emit_vt(b) for b in range(NB)]
            for b in range(NB):
                for qt in range(NQT):
                    emit_qtile(b, qt, vts[b],
                               nc.gpsimd if qt % 2 == 0 else nc.sync)
    nc.compile()
    return nc


def _get_nc():
    if "nc" not in _CACHE:
        _CACHE["nc"] = _build()
    return _CACHE["nc"]


def make_in_maps(Hn, A_stat, M_mask, Wq, Wk):
    Hn = np.ascontiguousarray(np.asarray(Hn, dtype=np.float32))
    A_stat = np.asarray(A_stat, dtype=np.float32)
    M_mask = np.asarray(M_mask)
    Wq = np.asarray(Wq, dtype=np.float32)
    Wk = np.asarray(Wk, dtype=np.float32)
    assert Hn.shape == (B_FULL, N, D)

    # G = Wq^T Wk / sqrt(D)  (nn.Linear applies x @ W.T; S = Q K^T/sqrt(D)
    # = Hn (Wq^T Wk) Hn^T / sqrt(D))
    G = ((Wq.T @ Wk) * SCALE).astype(np.float16)

    # E = mask * a/(1-a), a clipped: exp(s + eta*logit(a)) == exp(s)*a/(1-a)
    a = np.clip(A_stat, EPS, 1.0 - EPS)
    E = np.where(M_mask == 0, np.float32(0.0), a / (1.0 - a)).astype(np.float16)

    # [16, 256, 2048] transposed-node layout, fp16
    hnt_full = np.ascontiguousarray(Hn.astype(np.float16).transpose(0, 2, 1))

    in_maps = []
    for c in range(8):
        bg, qg = c // NQG, c % NQG
        bsl = slice(bg * NB, (bg + 1) * NB)
        qsl = slice(qg * NQ, (qg + 1) * NQ)
        # rotate key columns so this core's own query block sits at [0:NQ]
        hnt_c = np.roll(hnt_full[bsl], -qg * NQ, axis=2)
        e_c = np.roll(E[qsl], -qg * NQ, axis=1)
        in_maps.append({
            "hnt": np.ascontiguousarray(hnt_c),
            "g": G,
            "e": np.ascontiguousarray(e_c),
        })
    return in_maps


def assemble(results):
    out = np.empty((B_FULL, N, N), dtype=np.float32)
    for c in range(8):
        bg, qg = c // NQG, c % NQG
        bsl = slice(bg * NB, (bg + 1) * NB)
        qsl = slice(qg * NQ, (qg + 1) * NQ)
        o = np.asarray(results[c]["o"]).astype(np.float32)
        # un-rotate key columns (own block first -> global order)
        out[bsl, qsl, :] = np.roll(o, qg * NQ, axis=2)
    return out


def kernel(Hn, A_stat, M_mask, Wq, Wk):
    in_maps = make_in_maps(Hn, A_stat, M_mask, Wq, Wk)
    nc = _get_nc()
    res = bass_utils.run_bass_kernel_spmd(nc, in_maps, core_ids=list(range(8)))
    return assemble(res.results)


if __name__ == "__main__":
    rng = np.random.default_rng(0)
    inputs = {
        "Hn": rng.standard_normal((B_FULL, N, D), dtype=np.float32),
        "A_stat": rng.random((N, N), dtype=np.float32),
        "M_mask": rng.integers(0, 2, size=(N, N), dtype=np.int32),
        "Wq": rng.standard_normal((D, D), dtype=np.float32) / 16,
        "Wk": rng.standard_normal((D, D), dtype=np.float32) / 16,
    }
    out = kernel(**inputs)
    print(out.shape, out.dtype, out.sum())


# revision 8
# speedup vs baseline: 1.1330x; 1.1330x over previous
"""
Trainium2 Bass kernel for DynamicGraphAttention
(softmax(Hn Wq^T (Hn Wk^T)^T / sqrt(D) + eta*logit(clip(A)) masked)).

Shapes (hardcoded):
  Hn     [16, 2048, 256] f32
  A_stat [2048, 2048]    f32
  M_mask [2048, 2048]    int32
  Wq, Wk [256, 256]      f32
  out    [16, 2048, 2048] f32

Sharding across 8 NeuronCores: 4 batch-groups x 2 seq(query)-groups
(minimizes per-core input DMA: 4MB HnT + 4MB table). Core c handles
batches of group bg = c // 2 and query rows [qg*1024:(qg+1)*1024]
(qg = c % 2). Key columns are ROTATED per core so the core's own query
block is always columns [0:1024] of its HnT copy -- that keeps the
program SPMD-uniform; the host un-rotates output columns on assembly.

Math: S = Hn G Hn^T with G = Wq^T Wk / sqrt(D) (host-computed, fp16).
softmax bias b = eta*logit(clip(A)) masked to -100; E = exp(b).

Device pipeline per (batch b, 128-row q-tile qt):
  VT[b] = G^T HnT[b,:, :1024]      per batch (PE + ACT psum->fp16 copy)
  S     = VT^T HnT                 PSUM f32, 8 matmuls        (PE)
  two bias paths, balancing DVE against PE/ACT:
   qt in IDB_QTS (bias rows shipped in LOG domain):
    S    += I_128 @ btab[qt]       4 fp16 identity matmuls    (PE)
    u,rs  = exp(S), rowsum         ACT accumulator            (ACT)
   else (rows shipped as E = exp(b)):
    t     = exp(S)                 fp16                       (ACT)
    u,rs  = t*E, rowsum            scalar_tensor_tensor       (DVE)
  out   = u * (1/rs)               bf16 -> DMA                (DVE)

Output is shipped bf16 (rel err ~0.4% << 2e-2 gate) halving out DMA.
Per-core DMA: in 4+4+0.16MB, out 16.8MB  (baseline moved ~50MB).
"""

import math

import numpy as np

import concourse.bass as bass
import concourse.bacc as bacc
import concourse.tile as tile
from concourse import mybir
from concourse import bass_utils

F32 = mybir.dt.float32
BF16 = mybir.dt.bfloat16
FP16 = mybir.dt.float16

B_FULL = 16
N = 2048
D = 256
NBG = 4   # batch groups
NQG = 2   # seq (query-row) groups
NB = B_FULL // NBG        # batches per core = 4
NQ = N // NQG             # query rows per core = 1024
NQT = NQ // 128           # q tiles per core-batch = 8
IDB_QTS = (0, 4)          # q-tile rows whose bias rides the PE identity path
EPS = 1e-3
SCALE = 1.0 / math.sqrt(float(D))  # 1/16

_CACHE = {}


def _build():
    nc = bacc.Bacc("TRN2", debug=False, enable_asserts=False)

    hnt_d = nc.dram_tensor("hnt", [NB, D, N], FP16, kind="ExternalInput").ap()
    g_d = nc.dram_tensor("g", [D, D], FP16, kind="ExternalInput").ap()
    idb_d = nc.dram_tensor("idb", [128, 128], FP16, kind="ExternalInput").ap()
    tab_d = nc.dram_tensor("tab", [NQ, N], FP16, kind="ExternalInput").ap()
    o_d = nc.dram_tensor("o", [NB, NQ, N], BF16, kind="ExternalOutput").ap()

    with tile.TileContext(nc) as tc:
        with (
            tc.tile_pool(name="consts", bufs=1) as consts,
            tc.tile_pool(name="vtp", bufs=2) as vtp,
            tc.tile_pool(name="tp", bufs=3) as tp,
            tc.tile_pool(name="up", bufs=3) as up,
            tc.tile_pool(name="op", bufs=4) as op,
            tc.tile_pool(name="rsp", bufs=8) as rsp,
            tc.tile_pool(name="ps_s", bufs=2, space="PSUM") as ps_s,
        ):
            # ---- constants / persistent inputs ----
            # ordering matters: g + the q-halves of hnt feed VT[0] ASAP,
            # k-halves and tables stream in behind them.
            g_sb = consts.tile([128, 2, D], FP16, tag="g")
            nc.sync.dma_start(out=g_sb, in_=g_d.rearrange("(i p) e -> p i e", p=128))

            idb = consts.tile([128, 128], FP16, tag="idb")
            nc.gpsimd.dma_start(out=idb, in_=idb_d)

            hnt = [consts.tile([128, 2, N], FP16, tag=f"hnt{b}", name=f"hnt{b}")
                   for b in range(NB)]
            for b in range(NB):
                src = hnt_d[b].rearrange("(i p) n -> p i n", p=128)
                nc.sync.dma_start(out=hnt[b][:, :, 0:NQ], in_=src[:, :, 0:NQ])
                nc.sync.dma_start(out=hnt[b][:, :, NQ:N], in_=src[:, :, NQ:N])

            tab = []
            for t in range(NQT):
                e_t = consts.tile([128, N], FP16, tag=f"et{t}", name=f"et{t}")
                nc.gpsimd.dma_start(out=e_t, in_=tab_d[t * 128:(t + 1) * 128, :])
                tab.append(e_t)

            # ---- per-batch: VT = G^T HnT[:, :NQ]  ([128, 2*NQ] fp16,
            #      cols j*NQ+q hold (G^T HnT)[j*128+e, q]) ----
            def emit_vt(b):
                vp = ps_s.tile([128, N], F32, tag="s", name=f"vp{b}")
                for j in range(2):
                    for c in range(NQ // 512):
                        sl = slice(j * NQ + c * 512, j * NQ + c * 512 + 512)
                        qsl = slice(c * 512, (c + 1) * 512)
                        for i in range(2):
                            nc.tensor.matmul(
                                vp[:, sl],
                                lhsT=g_sb[:, i, j * 128:(j + 1) * 128],
                                rhs=hnt[b][:, i, qsl],
                                start=(i == 0),
                                stop=(i == 1),
                            )
                vt_b = vtp.tile([128, N], FP16, tag="vt", name=f"vt{b}")
                # psum->sbuf copy on ACT (Copy+Exp share one table set)
                nc.scalar.copy(out=vt_b, in_=vp)
                return vt_b

            def emit_qtile(b, qt, vt_b, out_queue):
                idb_path = qt in IDB_QTS
                qsl = slice(qt * 128, (qt + 1) * 128)
                s_ps = ps_s.tile([128, N], F32, tag="s", name=f"s{b}{qt}")
                for c in range(4):
                    csl = slice(c * 512, (c + 1) * 512)
                    for j in range(2):
                        nc.tensor.matmul(
                            s_ps[:, csl],
                            lhsT=vt_b[:, j * NQ + qt * 128:j * NQ + qt * 128 + 128],
                            rhs=hnt[b][:, j, csl],
                            start=(j == 0),
                            stop=(j == 1) and not idb_path,
                        )
                    if idb_path:
                        # S += I @ btab : log-domain bias via the PE
                        nc.tensor.matmul(
                            s_ps[:, csl], lhsT=idb, rhs=tab[qt][:, csl],
                            start=False, stop=True,
                        )
                u_sb = up.tile([128, N], FP16, tag="u", name=f"u{b}{qt}")
                rs = rsp.tile([128, 1], F32, tag="rs", name=f"rs{b}{qt}")
                if idb_path:
                    nc.scalar.activation(
                        out=u_sb, in_=s_ps,
                        func=mybir.ActivationFunctionType.Exp,
                        accum_out=rs,
                    )
                else:
                    t_sb = tp.tile([128, N], FP16, tag="t", name=f"t{b}{qt}")
                    nc.scalar.activation(
                        out=t_sb, in_=s_ps,
                        func=mybir.ActivationFunctionType.Exp,
                    )
                    # u = (t * 1.0) * E with fused row-sum accumulator.
                    # (InstTensorTensorReduce crashes TRN2 at runtime; this
                    # InstTensorScalarPtr form is HW-verified.)
                    nc.vector.scalar_tensor_tensor(
                        out=u_sb, in0=t_sb, scalar=1.0, in1=tab[qt],
                        op0=mybir.AluOpType.mult, op1=mybir.AluOpType.mult,
                        accum_out=rs,
                    )
                rinv = rsp.tile([128, 1], F32, tag="rinv", name=f"ri{b}{qt}")
                nc.vector.reciprocal(out=rinv, in_=rs)
                o_sb = op.tile([128, N], BF16, tag="o", name=f"o{b}{qt}")
                nc.vector.tensor_scalar(
                    out=o_sb, in0=u_sb, scalar1=rinv, scalar2=None,
                    op0=mybir.AluOpType.mult,
                )
                out_queue.dma_start(out=o_d[b, qsl, :], in_=o_sb)

            for b in range(NB):
                vt_b = emit_vt(b)
                for qt in range(NQT):
                    emit_qtile(b, qt, vt_b,
                               nc.gpsimd if qt % 2 == 0 else nc.sync)
    nc.compile()
    return nc


def _get_nc():
    if "nc" not in _CACHE:
        _CACHE["nc"] = _build()
    return _CACHE["nc"]


def make_in_maps(Hn, A_stat, M_mask, Wq, Wk):
    Hn = np.ascontiguousarray(np.asarray(Hn, dtype=np.float32))
    A_stat = np.asarray(A_stat, dtype=np.float32)
    M_mask = np.asarray(M_mask)
    Wq = np.asarray(Wq, dtype=np.float32)
    Wk = np.asarray(Wk, dtype=np.float32)
    assert Hn.shape == (B_FULL, N, D)

    # G = Wq^T Wk / sqrt(D)  (nn.Linear applies x @ W.T; S = Q K^T/sqrt(D)
    # = Hn (Wq^T Wk) Hn^T / sqrt(D))
    G = ((Wq.T @ Wk) * SCALE).astype(np.float16)
    idb = np.eye(128, dtype=np.float16)

    # bias b = logit(clip(a)), masked -> -100 (exp -> 0); E = a/(1-a) masked->0
    a = np.clip(A_stat, EPS, 1.0 - EPS)
    masked = M_mask == 0
    Btab = np.where(masked, np.float32(-100.0),
                    np.log(a) - np.log1p(-a)).astype(np.float16)
    E = np.where(masked, np.float32(0.0), a / (1.0 - a)).astype(np.float16)

    # [16, 256, 2048] transposed-node layout, fp16
    hnt_full = np.ascontiguousarray(Hn.astype(np.float16).transpose(0, 2, 1))

    in_maps = []
    for c in range(8):
        bg, qg = c // NQG, c % NQG
        bsl = slice(bg * NB, (bg + 1) * NB)
        qsl = slice(qg * NQ, (qg + 1) * NQ)
        # rotate key columns so this core's own query block sits at [0:NQ]
        hnt_c = np.roll(hnt_full[bsl], -qg * NQ, axis=2)
        # combined table: LOG-domain bias rows for the identity-matmul
        # q-tiles, E rows for the scalar_tensor_tensor q-tiles
        tab_c = np.roll(E[qsl], -qg * NQ, axis=1).copy()
        b_rows = np.roll(Btab[qsl], -qg * NQ, axis=1)
        for qt in IDB_QTS:
            r = slice(qt * 128, (qt + 1) * 128)
            tab_c[r] = b_rows[r]
        in_maps.append({
            "hnt": np.ascontiguousarray(hnt_c),
            "g": G,
            "idb": idb,
            "tab": np.ascontiguousarray(tab_c),
        })
    return in_maps


def assemble(results):
    out = np.empty((B_FULL, N, N), dtype=np.float32)
    for c in range(8):
        bg, qg = c // NQG, c % NQG
        bsl = slice(bg * NB, (bg + 1) * NB)
        qsl = slice(qg * NQ, (qg + 1) * NQ)
        o = np.asarray(results[c]["o"]).astype(np.float32)
        # un-rotate key columns (own block first -> global order)
        out[bsl, qsl, :] = np.roll(o, qg * NQ, axis=2)
    return out


def kernel(Hn, A_stat, M_mask, Wq, Wk):
    in_maps = make_in_maps(Hn, A_stat, M_mask, Wq, Wk)
    nc = _get_nc()
    res = bass_utils.run_bass_kernel_spmd(nc, in_maps, core_ids=list(range(8)))
    return assemble(res.results)


if __name__ == "__main__":
    rng = np.random.default_rng(0)
    inputs = {
        "Hn": rng.standard_normal((B_FULL, N, D), dtype=np.float32),
        "A_stat": rng.random((N, N), dtype=np.float32),
        "M_mask": rng.integers(0, 2, size=(N, N), dtype=np.int32),
        "Wq": rng.standard_normal((D, D), dtype=np.float32) / 16,
        "Wk": rng.standard_normal((D, D), dtype=np.float32) / 16,
    }
    out = kernel(**inputs)
    print(out.shape, out.dtype, out.sum())


# revision 11
# speedup vs baseline: 1.1989x; 1.0582x over previous
"""
Trainium2 Bass kernel for DynamicGraphAttention
(softmax(Hn Wq^T (Hn Wk^T)^T / sqrt(D) + eta*logit(clip(A)) masked)).

Shapes (hardcoded):
  Hn     [16, 2048, 256] f32
  A_stat [2048, 2048]    f32
  M_mask [2048, 2048]    int32
  Wq, Wk [256, 256]      f32
  out    [16, 2048, 2048] f32

Sharding across 8 NeuronCores: 4 batch-groups x 2 seq(query)-groups
(minimizes per-core input DMA: 4MB HnT + 4MB table). Core c handles
batches of group bg = c // 2 and query rows [qg*1024:(qg+1)*1024]
(qg = c % 2). Key columns are ROTATED per core so the core's own query
block is always columns [0:1024] of its HnT copy -- that keeps the
program SPMD-uniform; the host un-rotates output columns on assembly.

Math: S = Hn G Hn^T with G = Wq^T Wk / sqrt(D) (host-computed, fp16).
softmax bias b = eta*logit(clip(A)) masked to -100; E = exp(b).

Device pipeline per (batch b, 128-row q-tile qt):
  VT[b] = G^T HnT[b,:, :1024]      per batch (PE + ACT psum->fp16 copy)
  S     = VT^T HnT                 PSUM f32, 8 matmuls        (PE)
  two bias paths, balancing DVE against PE/ACT:
   qt in IDB_QTS (bias rows shipped in LOG domain):
    S    += I_128 @ btab[qt]       4 fp16 identity matmuls    (PE)
    u,rs  = exp(S), rowsum         ACT accumulator            (ACT)
   else (rows shipped as E = exp(b)):
    t     = exp(S)                 fp16                       (ACT)
    u,rs  = t*E, rowsum            scalar_tensor_tensor       (DVE)
  out   = u * (1/rs)               bf16 -> DMA                (DVE)

Output is shipped bf16 (rel err ~0.4% << 2e-2 gate) halving out DMA.
Per-core DMA: in 4+4+0.16MB, out 16.8MB  (baseline moved ~50MB).
"""

import math

import numpy as np

import concourse.bass as bass
import concourse.bacc as bacc
import concourse.tile as tile
from concourse import mybir
from concourse import bass_utils

F32 = mybir.dt.float32
BF16 = mybir.dt.bfloat16
FP16 = mybir.dt.float16

B_FULL = 16
N = 2048
D = 256
NBG = 4   # batch groups
NQG = 2   # seq (query-row) groups
NB = B_FULL // NBG        # batches per core = 4
NQ = N // NQG             # query rows per core = 1024
NQT = NQ // 128           # q tiles per core-batch = 8
IDB_QTS = (0, 4)          # q-tile rows whose bias rides the PE identity path
EPS = 1e-3
SCALE = 1.0 / math.sqrt(float(D))  # 1/16

_CACHE = {}


def _build():
    nc = bacc.Bacc("TRN2", debug=False, enable_asserts=False)

    hnt_d = nc.dram_tensor("hnt", [NB, D, N], FP16, kind="ExternalInput").ap()
    g_d = nc.dram_tensor("g", [D, D], FP16, kind="ExternalInput").ap()
    idb_d = nc.dram_tensor("idb", [128, 128], FP16, kind="ExternalInput").ap()
    tab_d = nc.dram_tensor("tab", [NQ, N], FP16, kind="ExternalInput").ap()
    o_d = nc.dram_tensor("o", [NB, NQ, N], BF16, kind="ExternalOutput").ap()

    with tile.TileContext(nc) as tc:
        with (
            tc.tile_pool(name="consts", bufs=1) as consts,
            tc.tile_pool(name="vtp", bufs=2) as vtp,
            tc.tile_pool(name="tp", bufs=3) as tp,
            tc.tile_pool(name="up", bufs=3) as up,
            tc.tile_pool(name="op", bufs=4) as op,
            tc.tile_pool(name="rsp", bufs=8) as rsp,
            tc.tile_pool(name="ps_s", bufs=2, space="PSUM") as ps_s,
        ):
            # ---- constants / persistent inputs ----
            # ordering matters: g + the q-halves of hnt feed VT[0] ASAP,
            # k-halves and tables stream in behind them.
            g_sb = consts.tile([128, 2, D], FP16, tag="g")
            nc.sync.dma_start(out=g_sb, in_=g_d.rearrange("(i p) e -> p i e", p=128))

            idb = consts.tile([128, 128], FP16, tag="idb")
            nc.gpsimd.dma_start(out=idb, in_=idb_d)

            hnt = [consts.tile([128, 2, N], FP16, tag=f"hnt{b}", name=f"hnt{b}")
                   for b in range(NB)]
            for b in range(NB):
                src = hnt_d[b].rearrange("(i p) n -> p i n", p=128)
                nc.sync.dma_start(out=hnt[b][:, :, 0:NQ], in_=src[:, :, 0:NQ])
                nc.sync.dma_start(out=hnt[b][:, :, NQ:N], in_=src[:, :, NQ:N])

            tab = []
            for t in range(NQT):
                e_t = consts.tile([128, N], FP16, tag=f"et{t}", name=f"et{t}")
                nc.gpsimd.dma_start(out=e_t, in_=tab_d[t * 128:(t + 1) * 128, :])
                tab.append(e_t)

            # ---- PE warmup: stream dummy matmuls on g while the first
            # hnt half loads, so VT[0] starts at a ramped p-state ----
            warm = ps_s.tile([128, N], F32, tag="s", name="warm")
            for w in range(16):
                nc.tensor.matmul(
                    warm[:, 0:D], lhsT=g_sb[:, 0, 0:128], rhs=g_sb[:, 1, :],
                    start=True, stop=True,
                )

            # ---- per-batch: VT = G^T HnT[:, :NQ]  ([128, 2*NQ] fp16,
            #      cols j*NQ+q hold (G^T HnT)[j*128+e, q]) ----
            def emit_vt(b):
                vp = ps_s.tile([128, N], F32, tag="s", name=f"vp{b}")
                for j in range(2):
                    for c in range(NQ // 512):
                        sl = slice(j * NQ + c * 512, j * NQ + c * 512 + 512)
                        qsl = slice(c * 512, (c + 1) * 512)
                        for i in range(2):
                            nc.tensor.matmul(
                                vp[:, sl],
                                lhsT=g_sb[:, i, j * 128:(j + 1) * 128],
                                rhs=hnt[b][:, i, qsl],
                                start=(i == 0),
                                stop=(i == 1),
                            )
                vt_b = vtp.tile([128, N], FP16, tag="vt", name=f"vt{b}")
                # psum->sbuf copy on ACT (Copy+Exp share one table set)
                nc.scalar.copy(out=vt_b, in_=vp)
                return vt_b

            def emit_qtile(b, qt, vt_b, out_queue):
                idb_path = qt in IDB_QTS
                qsl = slice(qt * 128, (qt + 1) * 128)
                s_ps = ps_s.tile([128, N], F32, tag="s", name=f"s{b}{qt}")
                for c in range(4):
                    csl = slice(c * 512, (c + 1) * 512)
                    for j in range(2):
                        nc.tensor.matmul(
                            s_ps[:, csl],
                            lhsT=vt_b[:, j * NQ + qt * 128:j * NQ + qt * 128 + 128],
                            rhs=hnt[b][:, j, csl],
                            start=(j == 0),
                            stop=(j == 1) and not idb_path,
                        )
                    if idb_path:
                        # S += I @ btab : log-domain bias via the PE
                        nc.tensor.matmul(
                            s_ps[:, csl], lhsT=idb, rhs=tab[qt][:, csl],
                            start=False, stop=True,
                        )
                u_sb = up.tile([128, N], FP16, tag="u", name=f"u{b}{qt}")
                rs = rsp.tile([128, 1], F32, tag="rs", name=f"rs{b}{qt}")
                if idb_path:
                    nc.scalar.activation(
                        out=u_sb, in_=s_ps,
                        func=mybir.ActivationFunctionType.Exp,
                        accum_out=rs,
                    )
                else:
                    t_sb = tp.tile([128, N], FP16, tag="t", name=f"t{b}{qt}")
                    nc.scalar.activation(
                        out=t_sb, in_=s_ps,
                        func=mybir.ActivationFunctionType.Exp,
                    )
                    # u = (t * 1.0) * E with fused row-sum accumulator.
                    # (InstTensorTensorReduce crashes TRN2 at runtime; this
                    # InstTensorScalarPtr form is HW-verified.)
                    nc.vector.scalar_tensor_tensor(
                        out=u_sb, in0=t_sb, scalar=1.0, in1=tab[qt],
                        op0=mybir.AluOpType.mult, op1=mybir.AluOpType.mult,
                        accum_out=rs,
                    )
                rinv = rsp.tile([128, 1], F32, tag="rinv", name=f"ri{b}{qt}")
                nc.vector.reciprocal(out=rinv, in_=rs)
                o_sb = op.tile([128, N], BF16, tag="o", name=f"o{b}{qt}")
                nc.vector.tensor_scalar(
                    out=o_sb, in0=u_sb, scalar1=rinv, scalar2=None,
                    op0=mybir.AluOpType.mult,
                )
                out_queue.dma_start(out=o_d[b, qsl, :], in_=o_sb)

            # VT[b+1] is emitted mid-batch (after qt=2) so the psum borrow
            # and the vt copy overlap the S stream instead of serializing
            # at the batch boundary.
            vts = {0: emit_vt(0)}
            for b in range(NB):
                for qt in range(NQT):
                    emit_qtile(b, qt, vts[b],
                               nc.gpsimd if qt % 2 == 0 else nc.sync)
                    if qt == 2 and b + 1 < NB:
                        vts[b + 1] = emit_vt(b + 1)
    nc.compile()
    return nc


def _get_nc():
    if "nc" not in _CACHE:
        _CACHE["nc"] = _build()
    return _CACHE["nc"]


def make_in_maps(Hn, A_stat, M_mask, Wq, Wk):
    Hn = np.ascontiguousarray(np.asarray(Hn, dtype=np.float32))
    A_stat = np.asarray(A_stat, dtype=np.float32)
    M_mask = np.asarray(M_mask)
    Wq = np.asarray(Wq, dtype=np.float32)
    Wk = np.asarray(Wk, dtype=np.float32)
    assert Hn.shape == (B_FULL, N, D)

    # G = Wq^T Wk / sqrt(D)  (nn.Linear applies x @ W.T; S = Q K^T/sqrt(D)
    # = Hn (Wq^T Wk) Hn^T / sqrt(D))
    G = ((Wq.T @ Wk) * SCALE).astype(np.float16)
    idb = np.eye(128, dtype=np.float16)

    # bias b = logit(clip(a)), masked -> -100 (exp -> 0); E = a/(1-a) masked->0
    a = np.clip(A_stat, EPS, 1.0 - EPS)
    masked = M_mask == 0
    Btab = np.where(masked, np.float32(-100.0),
                    np.log(a) - np.log1p(-a)).astype(np.float16)
    E = np.where(masked, np.float32(0.0), a / (1.0 - a)).astype(np.float16)

    # [16, 256, 2048] transposed-node layout, fp16
    hnt_full = np.ascontiguousarray(Hn.astype(np.float16).transpose(0, 2, 1))

    in_maps = []
    for c in range(8):
        bg, qg = c // NQG, c % NQG
        bsl = slice(bg * NB, (bg + 1) * NB)
        qsl = slice(qg * NQ, (qg + 1) * NQ)
        # rotate key columns so this core's own query block sits at [0:NQ]
        hnt_c = np.roll(hnt_full[bsl], -qg * NQ, axis=2)
        # combined table: LOG-domain bias rows for the identity-matmul
        # q-tiles, E rows for the scalar_tensor_tensor q-tiles
        tab_c = np.roll(E[qsl], -qg * NQ, axis=1).copy()
        b_rows = np.roll(Btab[qsl], -qg * NQ, axis=1)
        for qt in IDB_QTS:
            r = slice(qt * 128, (qt + 1) * 128)
            tab_c[r] = b_rows[r]
        in_maps.append({
            "hnt": np.ascontiguousarray(hnt_c),
            "g": G,
            "idb": idb,
            "tab": np.ascontiguousarray(tab_c),
        })
    return in_maps


def assemble(results):
    out = np.empty((B_FULL, N, N), dtype=np.float32)
    for c in range(8):
        bg, qg = c // NQG, c % NQG
        bsl = slice(bg * NB, (bg + 1) * NB)
        qsl = slice(qg * NQ, (qg + 1) * NQ)
        o = np.asarray(results[c]["o"]).astype(np.float32)
        # un-rotate key columns (own block first -> global order)
        out[bsl, qsl, :] = np.roll(o, qg * NQ, axis=2)
    return out


def kernel(Hn, A_stat, M_mask, Wq, Wk):
    in_maps = make_in_maps(Hn, A_stat, M_mask, Wq, Wk)
    nc = _get_nc()
    res = bass_utils.run_bass_kernel_spmd(nc, in_maps, core_ids=list(range(8)))
    return assemble(res.results)


if __name__ == "__main__":
    rng = np.random.default_rng(0)
    inputs = {
        "Hn": rng.standard_normal((B_FULL, N, D), dtype=np.float32),
        "A_stat": rng.random((N, N), dtype=np.float32),
        "M_mask": rng.integers(0, 2, size=(N, N), dtype=np.int32),
        "Wq": rng.standard_normal((D, D), dtype=np.float32) / 16,
        "Wk": rng.standard_normal((D, D), dtype=np.float32) / 16,
    }
    out = kernel(**inputs)
    print(out.shape, out.dtype, out.sum())


# revision 13
# speedup vs baseline: 1.2582x; 1.0494x over previous
"""
Trainium2 Bass kernel for DynamicGraphAttention
(softmax(Hn Wq^T (Hn Wk^T)^T / sqrt(D) + eta*logit(clip(A)) masked)).

Shapes (hardcoded):
  Hn     [16, 2048, 256] f32
  A_stat [2048, 2048]    f32
  M_mask [2048, 2048]    int32
  Wq, Wk [256, 256]      f32
  out    [16, 2048, 2048] f32

Sharding across 8 NeuronCores: 4 batch-groups x 2 seq(query)-groups
(minimizes per-core input DMA: 4MB HnT + 4MB table). Core c handles
batches of group bg = c // 2 and query rows [qg*1024:(qg+1)*1024]
(qg = c % 2). Key columns are ROTATED per core so the core's own query
block is always columns [0:1024] of its HnT copy -- that keeps the
program SPMD-uniform; the host un-rotates output columns on assembly.

Math: S = Hn G Hn^T with G = Wq^T Wk / sqrt(D) (host-computed, fp16).
softmax bias b = eta*logit(clip(A)) masked to -100; E = exp(b).

Device pipeline per (batch b, 128-row q-tile qt):
  VT[b] = G^T HnT[b,:, :1024]      per batch (PE + ACT psum->fp16 copy)
  S     = VT^T HnT                 PSUM f32, 8 matmuls        (PE)
  two bias paths, balancing DVE against PE/ACT:
   qt in IDB_QTS (bias rows shipped in LOG domain):
    S    += I_128 @ btab[qt]       4 fp16 identity matmuls    (PE)
    u,rs  = exp(S), rowsum         ACT accumulator            (ACT)
   else (rows shipped as E = exp(b)):
    t     = exp(S)                 fp16                       (ACT)
    u,rs  = t*E, rowsum            scalar_tensor_tensor       (DVE)
  out   = u * (1/rs)               bf16 -> DMA                (DVE)

Output is shipped bf16 (rel err ~0.4% << 2e-2 gate) halving out DMA.
Per-core DMA: in 4+4+0.16MB, out 16.8MB  (baseline moved ~50MB).
"""

import math

import numpy as np

import concourse.bass as bass
import concourse.bacc as bacc
import concourse.tile as tile
from concourse import mybir
from concourse import bass_utils

F32 = mybir.dt.float32
BF16 = mybir.dt.bfloat16
FP16 = mybir.dt.float16

B_FULL = 16
N = 2048
D = 256
NBG = 4   # batch groups
NQG = 2   # seq (query-row) groups
NB = B_FULL // NBG        # batches per core = 4
NQ = N // NQG             # query rows per core = 1024
NQT = NQ // 128           # q tiles per core-batch = 8
IDB_QTS = (0, 3, 6)       # q-tile rows whose bias rides the PE identity path
EPS = 1e-3
SCALE = 1.0 / math.sqrt(float(D))  # 1/16

_CACHE = {}


def _build():
    nc = bacc.Bacc("TRN2", debug=False, enable_asserts=False)

    hnt_d = nc.dram_tensor("hnt", [NB, D, N], FP16, kind="ExternalInput").ap()
    g_d = nc.dram_tensor("g", [D, D], FP16, kind="ExternalInput").ap()
    idb_d = nc.dram_tensor("idb", [128, 128], FP16, kind="ExternalInput").ap()
    tab_d = nc.dram_tensor("tab", [NQ, N], FP16, kind="ExternalInput").ap()
    o_d = nc.dram_tensor("o", [NB, NQ, N], BF16, kind="ExternalOutput").ap()

    with tile.TileContext(nc) as tc:
        with (
            tc.tile_pool(name="consts", bufs=1) as consts,
            tc.tile_pool(name="vtp", bufs=2) as vtp,
            tc.tile_pool(name="tp", bufs=3) as tp,
            tc.tile_pool(name="up", bufs=3) as up,
            tc.tile_pool(name="op", bufs=4) as op,
            tc.tile_pool(name="rsp", bufs=8) as rsp,
            tc.tile_pool(name="ps_s", bufs=2, space="PSUM") as ps_s,
        ):
            # ---- constants / persistent inputs ----
            # ordering matters: g + the q-halves of hnt feed VT[0] ASAP,
            # k-halves and tables stream in behind them.
            g_sb = consts.tile([128, 2, D], FP16, tag="g")
            nc.sync.dma_start(out=g_sb, in_=g_d.rearrange("(i p) e -> p i e", p=128))

            idb = consts.tile([128, 128], FP16, tag="idb")
            nc.gpsimd.dma_start(out=idb, in_=idb_d)

            hnt = [consts.tile([128, 2, N], FP16, tag=f"hnt{b}", name=f"hnt{b}")
                   for b in range(NB)]
            for b in range(NB):
                src = hnt_d[b].rearrange("(i p) n -> p i n", p=128)
                nc.sync.dma_start(out=hnt[b][:, :, 0:NQ], in_=src[:, :, 0:NQ])
                nc.sync.dma_start(out=hnt[b][:, :, NQ:N], in_=src[:, :, NQ:N])

            # tab[0:2] load early on the gpsimd queue (needed by the first
            # two q-tiles); tab[2:] queue on sync BEHIND the hnt loads so
            # their wire traffic cannot delay the critical first hnt half.
            tab = []
            for t in range(NQT):
                e_t = consts.tile([128, N], FP16, tag=f"et{t}", name=f"et{t}")
                if t < 2:
                    nc.gpsimd.dma_start(out=e_t, in_=tab_d[t * 128:(t + 1) * 128, :])
                tab.append(e_t)
            for t in range(2, NQT):
                nc.sync.dma_start(out=tab[t], in_=tab_d[t * 128:(t + 1) * 128, :])

            # ---- PE warmup: stream dummy matmuls on g while the first
            # hnt half loads, so VT[0] starts at a ramped p-state ----
            warm = ps_s.tile([128, N], F32, tag="s", name="warm")
            for w in range(16):
                nc.tensor.matmul(
                    warm[:, 0:D], lhsT=g_sb[:, 0, 0:128], rhs=g_sb[:, 1, :],
                    start=True, stop=True,
                )

            # ---- per-batch: VT = G^T HnT[:, :NQ]  ([128, 2*NQ] fp16,
            #      cols j*NQ+q hold (G^T HnT)[j*128+e, q]) ----
            def emit_vt(b):
                vp = ps_s.tile([128, N], F32, tag="s", name=f"vp{b}")
                for j in range(2):
                    for c in range(NQ // 512):
                        sl = slice(j * NQ + c * 512, j * NQ + c * 512 + 512)
                        qsl = slice(c * 512, (c + 1) * 512)
                        for i in range(2):
                            nc.tensor.matmul(
                                vp[:, sl],
                                lhsT=g_sb[:, i, j * 128:(j + 1) * 128],
                                rhs=hnt[b][:, i, qsl],
                                start=(i == 0),
                                stop=(i == 1),
                            )
                vt_b = vtp.tile([128, N], FP16, tag="vt", name=f"vt{b}")
                # psum->sbuf copy on ACT (Copy+Exp share one table set)
                nc.scalar.copy(out=vt_b, in_=vp)
                return vt_b

            def emit_qtile(b, qt, vt_b, out_queue):
                idb_path = qt in IDB_QTS
                qsl = slice(qt * 128, (qt + 1) * 128)
                s_ps = ps_s.tile([128, N], F32, tag="s", name=f"s{b}{qt}")
                for c in range(4):
                    csl = slice(c * 512, (c + 1) * 512)
                    for j in range(2):
                        nc.tensor.matmul(
                            s_ps[:, csl],
                            lhsT=vt_b[:, j * NQ + qt * 128:j * NQ + qt * 128 + 128],
                            rhs=hnt[b][:, j, csl],
                            start=(j == 0),
                            stop=(j == 1) and not idb_path,
                        )
                    if idb_path:
                        # S += I @ btab : log-domain bias via the PE
                        nc.tensor.matmul(
                            s_ps[:, csl], lhsT=idb, rhs=tab[qt][:, csl],
                            start=False, stop=True,
                        )
                u_sb = up.tile([128, N], FP16, tag="u", name=f"u{b}{qt}")
                rs = rsp.tile([128, 1], F32, tag="rs", name=f"rs{b}{qt}")
                if idb_path:
                    nc.scalar.activation(
                        out=u_sb, in_=s_ps,
                        func=mybir.ActivationFunctionType.Exp,
                        accum_out=rs,
                    )
                else:
                    t_sb = tp.tile([128, N], FP16, tag="t", name=f"t{b}{qt}")
                    nc.scalar.activation(
                        out=t_sb, in_=s_ps,
                        func=mybir.ActivationFunctionType.Exp,
                    )
                    # u = (t * 1.0) * E with fused row-sum accumulator.
                    # (InstTensorTensorReduce crashes TRN2 at runtime; this
                    # InstTensorScalarPtr form is HW-verified.)
                    nc.vector.scalar_tensor_tensor(
                        out=u_sb, in0=t_sb, scalar=1.0, in1=tab[qt],
                        op0=mybir.AluOpType.mult, op1=mybir.AluOpType.mult,
                        accum_out=rs,
                    )
                rinv = rsp.tile([128, 1], F32, tag="rinv", name=f"ri{b}{qt}")
                nc.vector.reciprocal(out=rinv, in_=rs)
                o_sb = op.tile([128, N], BF16, tag="o", name=f"o{b}{qt}")
                nc.vector.tensor_scalar(
                    out=o_sb, in0=u_sb, scalar1=rinv, scalar2=None,
                    op0=mybir.AluOpType.mult,
                )
                out_queue.dma_start(out=o_d[b, qsl, :], in_=o_sb)

            # VT[b+1] is emitted mid-batch (after qt=2) so the psum borrow
            # and the vt copy overlap the S stream instead of serializing
            # at the batch boundary.
            vts = {0: emit_vt(0)}
            for b in range(NB):
                for qt in range(NQT):
                    emit_qtile(b, qt, vts[b],
                               nc.gpsimd if qt % 2 == 0 else nc.sync)
                    if qt == 2 and b + 1 < NB:
                        vts[b + 1] = emit_vt(b + 1)
    nc.compile()
    return nc


def _get_nc():
    if "nc" not in _CACHE:
        _CACHE["nc"] = _build()
    return _CACHE["nc"]


def make_in_maps(Hn, A_stat, M_mask, Wq, Wk):
    Hn = np.ascontiguousarray(np.asarray(Hn, dtype=np.float32))
    A_stat = np.asarray(A_stat, dtype=np.float32)
    M_mask = np.asarray(M_mask)
    Wq = np.asarray(Wq, dtype=np.float32)
    Wk = np.asarray(Wk, dtype=np.float32)
    assert Hn.shape == (B_FULL, N, D)

    # G = Wq^T Wk / sqrt(D)  (nn.Linear applies x @ W.T; S = Q K^T/sqrt(D)
    # = Hn (Wq^T Wk) Hn^T / sqrt(D))
    G = ((Wq.T @ Wk) * SCALE).astype(np.float16)
    idb = np.eye(128, dtype=np.float16)

    # bias b = logit(clip(a)), masked -> -100 (exp -> 0); E = a/(1-a) masked->0
    a = np.clip(A_stat, EPS, 1.0 - EPS)
    masked = M_mask == 0
    Btab = np.where(masked, np.float32(-100.0),
                    np.log(a) - np.log1p(-a)).astype(np.float16)
    E = np.where(masked, np.float32(0.0), a / (1.0 - a)).astype(np.float16)

    # [16, 256, 2048] transposed-node layout, fp16
    hnt_full = np.ascontiguousarray(Hn.astype(np.float16).transpose(0, 2, 1))

    in_maps = []
    for c in range(8):
        bg, qg = c // NQG, c % NQG
        bsl = slice(bg * NB, (bg + 1) * NB)
        qsl = slice(qg * NQ, (qg + 1) * NQ)
        # rotate key columns so this core's own query block sits at [0:NQ]
        hnt_c = np.roll(hnt_full[bsl], -qg * NQ, axis=2)
        # combined table: LOG-domain bias rows for the identity-matmul
        # q-tiles, E rows for the scalar_tensor_tensor q-tiles
        tab_c = np.roll(E[qsl], -qg * NQ, axis=1).copy()
        b_rows = np.roll(Btab[qsl], -qg * NQ, axis=1)
        for qt in IDB_QTS:
            r = slice(qt * 128, (qt + 1) * 128)
            tab_c[r] = b_rows[r]
        in_maps.append({
            "hnt": np.ascontiguousarray(hnt_c),
            "g": G,
            "idb": idb,
            "tab": np.ascontiguousarray(tab_c),
        })
    return in_maps


def assemble(results):
    out = np.empty((B_FULL, N, N), dtype=np.float32)
    for c in range(8):
        bg, qg = c // NQG, c % NQG
        bsl = slice(bg * NB, (bg + 1) * NB)
        qsl = slice(qg * NQ, (qg + 1) * NQ)
        o = np.asarray(results[c]["o"]).astype(np.float32)
        # un-rotate key columns (own block first -> global order)
        out[bsl, qsl, :] = np.roll(o, qg * NQ, axis=2)
    return out


def kernel(Hn, A_stat, M_mask, Wq, Wk):
    in_maps = make_in_maps(Hn, A_stat, M_mask, Wq, Wk)
    nc = _get_nc()
    res = bass_utils.run_bass_kernel_spmd(nc, in_maps, core_ids=list(range(8)))
    return assemble(res.results)


if __name__ == "__main__":
    rng = np.random.default_rng(0)
    inputs = {
        "Hn": rng.standard_normal((B_FULL, N, D), dtype=np.float32),
        "A_stat": rng.random((N, N), dtype=np.float32),
        "M_mask": rng.integers(0, 2, size=(N, N), dtype=np.int32),
        "Wq": rng.standard_normal((D, D), dtype=np.float32) / 16,
        "Wk": rng.standard_normal((D, D), dtype=np.float32) / 16,
    }
    out = kernel(**inputs)
    print(out.shape, out.dtype, out.sum())
